# revision 44
# baseline (speedup 1.0000x reference)
"""Multi-head attention (B=4, S=2048, D=1024, H=16) on 8 trn2 NeuronCores.

Sharding: data-parallel over batch (4) x tensor-parallel over heads (2 groups
of 8 heads).  Core c handles batch b=c//2, head group g=c%2: it gets
Wq/Wk/Wv[:, g*512:(g+1)*512] and Wo[g*512:(g+1)*512, :] and produces a partial
output [S, D]; the host sums the two partials of each batch (the row-split of
Wo makes the full output an exact sum of the two group partials).

Per-core kernel.  bf16 operands everywhere (fp32 PSUM accumulation; rel err
~5e-3 vs the fp32 reference, gate is 2e-2): halves LDWEIGHTS (FWL), SBUF and
DMA traffic.  The kernel is ACT(exp)-bound: 33.5M softmax exps/core at
1 elem/lane/cycle @1.2GHz is a ~285us floor, so everything else is scheduled
to hide inside the EXP stream:

  1. xT [D, S] comes pre-transposed from the host (sharding-time prep).
  2. V = x @ wv first (PV stationaries), then K/Q projections are emitted
     JUST-IN-TIME: K ct / Q (ct, qc) chains are injected between attention
     kt-slots of the preceding block, so the ACT engine starts exp-ing
     ~35us in and never drains.
  3. attention (qc, pt) blocks, head PAIRS: the two heads of a partition
     tile run their K=64 scoresT matmuls on PE row groups (0,0)/(64,0) via
     tile_position; one ACT instr does exp(s/8 + maskbias) for both heads;
     PV in transposed form outT[65, q] += V_h(+ones).T @ expT accumulates
     values + softmax denominators (ones column in an M=128 stationary
     window).  Per q-chunk: reciprocal on DVE (reciprocal_approx_fast),
     DRAM-bounce partition-broadcast, in-place normalize, then y = outT.T
     @ wo in 4 pieces -- all injected into later blocks' kt-slot gaps.
"""

import os
import sys

import ml_dtypes
import numpy as np

_TRN_REPO = "/opt/trn_rl_repo"
if _TRN_REPO not in sys.path:
    sys.path.insert(0, _TRN_REPO)

from contextlib import ExitStack

import concourse.bass as bass
import concourse.mybir as mybir
import concourse.tile as tile
from concourse import library_config
from concourse.bass_utils import run_bass_kernel_spmd

# If BASS_TRACE is set in the environment, run_bass_kernel_spmd imports
# antenv.axon_hooks, which this container image lacks -- pre-install a stub
# so kernel() degrades to an untraced run instead of crashing.  test.py
# overwrites the stub with a real ctypes-backed hook for profiling.
if "antenv.axon_hooks" not in sys.modules:
    try:
        import antenv.axon_hooks  # noqa: F401
    except Exception:
        import types as _types

        _hookmod = _types.ModuleType("antenv.axon_hooks")
        _hookstore = {}
        _hookmod.set_axon_ntff_profile_hook = lambda h: _hookstore.__setitem__(
            "h", h
        )
        _hookmod.get_axon_ntff_profile_hook = lambda: _hookstore.get("h")
        sys.modules["antenv.axon_hooks"] = _hookmod
        try:
            import antenv

            antenv.axon_hooks = _hookmod
        except Exception:
            pass

S, D, H, DK = 2048, 1024, 16, 64
NCORES = 8
HG = 2                # head-parallel groups
B = 4                 # batches
H8 = H // HG          # heads per core
C = H8 * DK           # 512: per-core projection width
P = 128
KT = D // P           # 8  k-tiles over D
ST = S // P           # 16 tiles over S
CT = C // P           # 4  tiles over C
VW = DK + 1           # 65: v columns + ones column
QC = 512              # q-chunk in attention phase (head-pair scheme)
NQC = S // QC

f32 = mybir.dt.float32
f32r = mybir.dt.float32r
bf16 = mybir.dt.bfloat16
i32 = mybir.dt.int32
FT = mybir.ActivationFunctionType
ALU = mybir.AluOpType


def build_nc(split_waits=True):
    nc = bass.Bass()
    xt_d = nc.declare_dram_parameter("xt", [D, S], bf16, isOutput=False)
    wq_d = nc.declare_dram_parameter("wq", [D, C], bf16, isOutput=False)
    wk_d = nc.declare_dram_parameter("wk", [D, C], bf16, isOutput=False)
    wv_d = nc.declare_dram_parameter("wv", [D, C], bf16, isOutput=False)
    wo_d = nc.declare_dram_parameter("wo", [C, D], bf16, isOutput=False)
    mask_d = nc.declare_dram_parameter("maskt", [P, ST], i32, isOutput=False)
    y_d = nc.declare_dram_parameter("y", [S, D], f32, isOutput=True)

    with tile.TileContext(nc) as tc, ExitStack() as ctx:
        perm = ctx.enter_context(tc.tile_pool(name="perm", bufs=1))

        # mask bias: (m - 1) * 1e9 per key, keys on partitions, one col per k-tile
        mask_i = perm.tile([P, ST], i32)
        nc.sync.dma_start(mask_i, mask_d[:, :])
        mask_b = perm.tile([P, ST], f32)
        nc.vector.tensor_copy(mask_b, mask_i)
        nc.vector.tensor_scalar(mask_b, mask_b, -1.0, 1.0e9, ALU.add, ALU.mult)

        # xT arrives in 8 S-chunks so the V projection can start after ~1/8
        # of the transfer (chunks land on different DMA queues).
        xT = perm.tile([P, KT, S], bf16)
        xt_r = xt_d.rearrange("(kt p) s -> p kt s", p=P)
        # wk first (the critical lead-in path), then the first xT s-range,
        # then the rest; everything in small chunks so they spread across
        # the 16 DMA queues (~11 GB/s each).
        wv_sb = perm.tile([P, KT, C], bf16)
        wq_sb = perm.tile([P, KT, C], bf16)
        wk_sb = perm.tile([P, KT, C], bf16)
        # critical prefix on disjoint queues: wk (8 chunks) || xT s0:512
        # (8 chunks) feed the first K/Q chains; then wq, wv, the rest of xT.
        wk_r = wk_d.rearrange("(kt p) c -> p kt c", p=P)
        for kt in range(KT):
            nc.sync.dma_start(
                wk_sb[:, kt : kt + 1, :], wk_r[:, kt : kt + 1, :]
            )
        for i in range(4):
            for j in range(2):
                nc.sync.dma_start(
                    xT[:, 2 * i : 2 * (i + 1), 256 * j : 256 * (j + 1)],
                    xt_r[:, 2 * i : 2 * (i + 1), 256 * j : 256 * (j + 1)],
                )
        wq_r = wq_d.rearrange("(kt p) c -> p kt c", p=P)
        for kt in range(KT):
            nc.sync.dma_start(
                wq_sb[:, kt : kt + 1, :], wq_r[:, kt : kt + 1, :]
            )
        wv_r = wv_d.rearrange("(kt p) c -> p kt c", p=P)
        for kt in range(KT):
            nc.sync.dma_start(
                wv_sb[:, kt : kt + 1, :], wv_r[:, kt : kt + 1, :]
            )
        for i in range(2):
            for j in range(2, 8):
                nc.sync.dma_start(
                    xT[:, 4 * i : 4 * (i + 1), 256 * j : 256 * (j + 1)],
                    xt_r[:, 4 * i : 4 * (i + 1), 256 * j : 256 * (j + 1)],
                )
        wo_sb = perm.tile([P, CT, D], bf16)
        wo_r = wo_d.rearrange("(pt p) e -> p pt e", p=P)
        for pt in range(CT):
            nc.sync.dma_start(wo_sb[:, pt : pt + 1, :], wo_r[:, pt : pt + 1, :])

        QT = perm.tile([P, CT, S], bf16)
        KTl = perm.tile([P, CT, S], bf16)
        # per-head PV stationary [V_h | ones]: M=65 keeps LDWEIGHTS short
        # (cost scales with stationary columns)
        V = perm.tile([P, ST, H8 * VW], bf16)
        V4 = V.rearrange("p st (h w) -> p st h w", w=VW)
        ones_sc = perm.tile([P, 1], f32)
        nc.vector.memset(ones_sc[:, :], 1.0)
        for st in range(ST):
            nc.vector.tensor_copy(
                V4[:, st, :, DK : DK + 1],
                ones_sc[:, :, None].to_broadcast((P, H8, 1)),
            )
        outT = perm.tile([P, CT, S], bf16)
        # 32 (head, q-chunk) row-sum vectors packed at start partitions
        # {0,32,64,96} x 8 column blocks (engine SBUF APs must start at k*32)
        rowsums = perm.tile([P, H8 * NQC // 4, QC], f32)
        nc.vector.memset(rowsums[:, :, :], 1.0)
        # bf16 copies of 1/rowsums (stationary/moving for the PE-matmul
        # partition-broadcast) + an all-ones bf16 column
        rowsums_bf = perm.tile([P, H8 * NQC // 4, QC], bf16)
        ones_bf = perm.tile([P, DK], bf16)
        nc.vector.tensor_copy(ones_bf, ones_sc.to_broadcast((P, DK)))

        with (
            tc.tile_pool(name="scps", bufs=2, space="PSUM") as scp,
            tc.tile_pool(name="otps", bufs=2, space="PSUM") as otp,
            tc.tile_pool(name="auxps", bufs=2, space="PSUM") as aux,
            tc.tile_pool(name="expool", bufs=5) as exp_pool,
            tc.tile_pool(name="ypool", bufs=3) as ypl,
        ):
            # warm the PE (HAM un-throttles after ~3.4us of activity) during
            # the input-DMA wait with throwaway matmuls on a zeroed tile
            warm = exp_pool.tile([P, 512], bf16, tag="warm")
            nc.gpsimd.memset(warm[:, :], 0.0)
            warm_ps = aux.tile([P, 512], f32, tag="mm", name="warmps")
            for _ in range(16):
                nc.tensor.matmul(
                    warm_ps, warm[:, 0:P], warm, start=True, stop=True
                )

            # projection chains accumulate even/odd kt in two PSUM tiles
            # (banks alternate, so each LDWEIGHTS pulls ahead during the
            # other bank's matmul) and combine with one DVE add at evac.
            def v_block(st):
                psa = aux.tile([P, C], f32, tag="mm", name="psva")
                psb = aux.tile([P, C], f32, tag="mm", name="psvb")
                for kt in range(KT):
                    nc.tensor.matmul(
                        psa if kt % 2 == 0 else psb,
                        xT[:, kt, st * P : (st + 1) * P],
                        wv_sb[:, kt, :],
                        start=(kt < 2),
                        stop=(kt >= KT - 2),
                    )
                cmb = exp_pool.tile([P, C], bf16, tag="cmb", name="cmbv")
                nc.vector.tensor_copy(cmb, psb)
                nc.vector.tensor_add(
                    V4[:, st, :, 0:DK],
                    psa.rearrange("p (h w) -> p h w", w=DK),
                    cmb.rearrange("p (h w) -> p h w", w=DK),
                )

            def proj_chain(w_sb, dst, ct, sch):
                psa = aux.tile([P, C], f32, tag="mm", name="pspa")
                psb = aux.tile([P, C], f32, tag="mm", name="pspb")
                for kt in range(KT):
                    nc.tensor.matmul(
                        psa if kt % 2 == 0 else psb,
                        w_sb[:, kt, ct * P : (ct + 1) * P],
                        xT[:, kt, sch * 512 : (sch + 1) * 512],
                        start=(kt < 2),
                        stop=(kt >= KT - 2),
                    )
                cmb = exp_pool.tile([P, C], bf16, tag="cmb", name="cmbp")
                nc.vector.tensor_copy(cmb, psb)
                nc.vector.tensor_add(
                    dst[:, ct, sch * 512 : (sch + 1) * 512], psa, cmb
                )

            def norm_half(qc, half):
                # 1/rowsums via exp(-ln(x)) on ACT (unused lanes hold 1.0),
                # partition-broadcast via K=1 PE matmuls (ones column x recip
                # row -> PSUM), then normalize 2 of the 4 partition tiles of
                # q-chunk qc of outT in place (half 0: heads 0-3, half 1:
                # heads 4-7 -- lets the last chunk's first half run early).
                qs = slice(qc * QC, (qc + 1) * QC)
                blk = 2 * qc + half
                rsp = rowsums[:, blk : blk + 1, :]
                nc.scalar.activation(rsp, rsp, FT.Ln)
                nc.scalar.activation(rsp, rsp, FT.Exp, scale=-1.0)
                nc.vector.tensor_copy(rowsums_bf[:, blk : blk + 1, :], rsp)
                for pt in (2 * half, 2 * half + 1):
                    bc = aux.tile([P, QC], f32, tag="mm", name="bc")
                    for hh in range(2):
                        h = 2 * pt + hh
                        r0 = (h % 4) * 32
                        nc.tensor.matmul(
                            bc[hh * DK : (hh + 1) * DK, :],
                            ones_bf[r0 : r0 + 1, :],
                            rowsums_bf[r0 : r0 + 1, blk, :],
                            start=True,
                            stop=True,
                            tile_position=(r0, hh * DK),
                        )
                    nc.vector.tensor_mul(outT[:, pt, qs], outT[:, pt, qs], bc)

            def y_piece(qc, sti):
                # one S-tile of y = outT.T @ wo; pt-outer so each outT
                # stationary serves both 512-wide output halves.
                st = qc * (QC // P) + sti
                y_sb = ypl.tile([P, D], f32, tag="y")
                pss = [
                    aux.tile([P, QC], f32, tag="mm", name=f"rsy{e}")
                    for e in range(2)
                ]
                for pt in range(CT):
                    for ec in range(2):
                        nc.tensor.matmul(
                            pss[ec],
                            outT[:, pt, st * P : (st + 1) * P],
                            wo_sb[:, pt, ec * 512 : (ec + 1) * 512],
                            start=(pt == 0),
                            stop=(pt == CT - 1),
                        )
                for ec in range(2):
                    nc.vector.tensor_copy(
                        y_sb[:, ec * 512 : (ec + 1) * 512], pss[ec]
                    )
                    nc.sync.dma_start(
                        y_d[st * P : (st + 1) * P, ec * 512 : (ec + 1) * 512],
                        y_sb[:, ec * 512 : (ec + 1) * 512],
                    )

            # cross-block software pipeline: each slot's PV pair is deferred
            # until after the NEXT slot's scores+exp, so at block boundaries
            # the next block's first exp is not stuck behind PVs that wait
            # on this block's last exp.
            pending_pv = [None]

            def flush_pv():
                if pending_pv[0] is not None:
                    fn, pending_pv[0] = pending_pv[0], None
                    fn()

            def att_block(qc, pt, work):
                # 16 kt-slots of scores-pair -> exp -> PV(prev slot); `work`
                # maps kt-slot -> closures (projection chains / V blocks /
                # norm / y pieces for neighboring blocks) injected so the PE
                # executes them inside the ACT-bound exp stream.
                qs = slice(qc * QC, (qc + 1) * QC)
                h0, h1 = 2 * pt, 2 * pt + 1
                ot0 = otp.tile([VW, QC], f32, tag="ot", name="ot0")
                ot1 = otp.tile([VW, QC], f32, tag="ot", name="ot1")

                def drain():
                    # rowsum vector (h, qc) at row (h%4)*32, block qc*2+h//4
                    for half, ot in ((0, ot0), (1, ot1)):
                        h = 2 * pt + half
                        nc.vector.tensor_copy(
                            rowsums[
                                (h % 4) * 32 : (h % 4) * 32 + 1,
                                2 * qc + h // 4,
                                :,
                            ],
                            ot[DK : DK + 1, :],
                        )
                        nc.vector.tensor_copy(
                            outT[half * DK : (half + 1) * DK, pt, qs],
                            ot[0:DK, :],
                        )

                for kt in range(ST):
                    for fn in work.get(kt, ()):
                        fn()
                    sc_ps = scp.tile([P, 2, QC], f32, tag="sc")
                    nc.tensor.matmul(
                        sc_ps[:, 0, :],
                        KTl[0:DK, pt, kt * P : (kt + 1) * P],
                        QT[0:DK, pt, qs],
                        start=True,
                        stop=True,
                        tile_position=(0, 0),
                    )
                    nc.tensor.matmul(
                        sc_ps[:, 1, :],
                        KTl[DK:P, pt, kt * P : (kt + 1) * P],
                        QT[DK:P, pt, qs],
                        start=True,
                        stop=True,
                        tile_position=(64, 0),
                    )
                    ex = exp_pool.tile([P, 2, QC], bf16, tag="ex")
                    nc.scalar.activation(
                        ex.rearrange("p a b -> p (a b)"),
                        sc_ps.rearrange("p a b -> p (a b)"),
                        FT.Exp,
                        bias=mask_b[:, kt : kt + 1],
                    )
                    flush_pv()

                    def pv(kt=kt, ex=ex):
                        nc.tensor.matmul(
                            ot0,
                            V4[:, kt, h0, :],
                            ex[:, 0, :],
                            start=(kt == 0),
                            stop=(kt == ST - 1),
                        )
                        nc.tensor.matmul(
                            ot1,
                            V4[:, kt, h1, :],
                            ex[:, 1, :],
                            start=(kt == 0),
                            stop=(kt == ST - 1),
                        )
                        if kt == ST - 1:
                            drain()

                    pending_pv[0] = pv

            # ---- attention with just-in-time V/K/Q projections + y.
            # Lead-in is only K(ct0, sch0) + Q(0, 0): the first attention
            # block's early kt-slots run on those while everything else
            # (V blocks, remaining K/Q chains) is injected into slot gaps.
            def K_chain(ct, sch):
                return lambda: proj_chain(wk_sb, KTl, ct, sch)

            def Q_chain(ct, qc):
                return lambda: proj_chain(wq_sb, QT, ct, qc)

            def V_blk(st):
                return lambda: v_block(st)

            proj_chain(wk_sb, KTl, 0, 0)
            proj_chain(wq_sb, QT, 0, 0)

            for qc in range(NQC):
                for pt in range(CT):
                    work = {}

                    def add(kt, fn):
                        work.setdefault(kt, []).append(fn)

                    if qc == 0:
                        if pt == 0:
                            # V tile st must land before its PV, which is
                            # deferred past exp(st+1): inject one slot later
                            # so each slot's scores lead the PE queue
                            for st in range(ST):
                                add(min(st + 1, ST - 1), V_blk(st))
                            add(1, K_chain(0, 1))
                            add(5, K_chain(0, 2))
                            add(9, K_chain(0, 3))
                            add(12, K_chain(1, 0))
                            add(14, Q_chain(1, 0))
                        elif pt < 3:
                            add(1, K_chain(pt, 1))
                            add(4, K_chain(pt, 2))
                            add(7, K_chain(pt, 3))
                            add(10, K_chain(pt + 1, 0))
                            add(13, Q_chain(pt + 1, 0))
                        else:
                            add(1, K_chain(3, 1))
                            add(4, K_chain(3, 2))
                            add(7, K_chain(3, 3))
                            add(11, Q_chain(0, 1))
                    else:
                        if pt == 0:
                            add(1, lambda q=qc - 1: norm_half(q, 0))
                            add(3, lambda q=qc - 1: norm_half(q, 1))
                            add(8, lambda q=qc - 1: y_piece(q, 0))
                            add(12, Q_chain(1, qc))
                        elif pt == 1:
                            add(2, lambda q=qc - 1: y_piece(q, 1))
                            add(8, lambda q=qc - 1: y_piece(q, 2))
                            add(12, Q_chain(2, qc))
                        elif pt == 2:
                            add(2, lambda q=qc - 1: y_piece(q, 3))
                            if qc == NQC - 1:
                                # last chunk's heads 0-3 normalize early
                                # (their rowsums completed with block pt1)
                                add(6, lambda: norm_half(NQC - 1, 0))
                            add(12, Q_chain(3, qc))
                        elif qc < NQC - 1:
                            add(8, Q_chain(0, qc + 1))
                    att_block(qc, pt, work)

            flush_pv()
            norm_half(NQC - 1, 1)
            for sti in range(4):
                y_piece(NQC - 1, sti)

    if split_waits:
        _split_matmul_waits(nc)
    return nc


def _split_matmul_waits(nc):
    """fp32/f32r matmuls (and DMA descriptors) lower to structs that hold
    only ONE sync wait; move extra waits onto a nop on the same engine."""
    import bass_rust

    n = 0
    for f in nc.m.functions:
        for blk in f.blocks:
            out = []
            for inst in blk.instructions:
                si = getattr(inst, "sync_info", None)
                if si is not None and len(si.on_wait) > 1:
                    waits = list(si.on_wait)
                    for w in waits[:-1]:
                        nop = bass_rust.InstNoOp(
                            name=f"I-mmw{n}", ins=[], outs=[], engine=inst.engine
                        )
                        n += 1
                        nop.sync_info = bass_rust.SyncInfo(
                            on_wait=[w], on_update=[]
                        )
                        out.append(nop)
                    inst.sync_info = bass_rust.SyncInfo(
                        on_wait=waits[-1:], on_update=list(si.on_update)
                    )
                out.append(inst)
            blk.instructions = out
    return nc


_NC_CACHE = None


def get_nc():
    global _NC_CACHE
    if _NC_CACHE is None:
        _NC_CACHE = build_nc()
    return _NC_CACHE


def make_in_maps(inputs):
    inp = np.asarray(inputs["inputs"], dtype=np.float32)
    mask = np.asarray(inputs["mask"], dtype=np.int32)
    Wq = np.asarray(inputs["Wq"], dtype=np.float32)
    Wk = np.asarray(inputs["Wk"], dtype=np.float32)
    Wv = np.asarray(inputs["Wv"], dtype=np.float32)
    Wo = np.asarray(inputs["Wo"], dtype=np.float32)
    b16 = ml_dtypes.bfloat16

    in_maps = []
    for c in range(NCORES):
        b, g = c // HG, c % HG
        cs = slice(g * C, (g + 1) * C)
        in_maps.append(
            {
                "xt": np.ascontiguousarray(inp[b].T.astype(b16)),
                "wq": np.ascontiguousarray((Wq[:, cs] * 0.125).astype(b16)),
                "wk": np.ascontiguousarray(Wk[:, cs].astype(b16)),
                "wv": np.ascontiguousarray(Wv[:, cs].astype(b16)),
                "wo": np.ascontiguousarray(Wo[cs, :].astype(b16)),
                "maskt": np.ascontiguousarray(mask[b].reshape(ST, P).T),
            }
        )
    return in_maps


def gather(results):
    out = np.empty((B, S, D), np.float32)
    for b in range(B):
        out[b] = results[HG * b]["y"] + results[HG * b + 1]["y"]
    return out


def run(inputs, **kwargs):
    """Run on hardware; returns (output, BassKernelResults)."""
    res = run_bass_kernel_spmd(
        get_nc(), make_in_maps(inputs), list(range(NCORES)), **kwargs
    )
    return gather(res.results), res


def kernel(**inputs) -> np.ndarray:
    out, _ = run(inputs)
    return out


# revision 45
# speedup vs baseline: 1.2430x; 1.2430x over previous
"""Multi-head attention (B=4, S=2048, D=1024, H=16) on 8 trn2 NeuronCores.

Sharding: data-parallel over batch (4) x tensor-parallel over heads (2 groups
of 8 heads).  Core c handles batch b=c//2, head group g=c%2: it gets
Wq/Wk/Wv[:, g*512:(g+1)*512] and Wo[g*512:(g+1)*512, :] and produces a partial
output [S, D]; the host sums the two partials of each batch (the row-split of
Wo makes the full output an exact sum of the two group partials).

Per-core kernel.  bf16 operands everywhere (fp32 PSUM accumulation; rel err
~5e-3 vs the fp32 reference, gate is 2e-2): halves LDWEIGHTS (FWL), SBUF and
DMA traffic.  The kernel is ACT(exp)-bound: 33.5M softmax exps/core at
1 elem/lane/cycle @1.2GHz is a ~285us floor, so everything else is scheduled
to hide inside the EXP stream:

  1. xT [D, S] comes pre-transposed from the host (sharding-time prep).
  2. V = x @ wv first (PV stationaries), then K/Q projections are emitted
     JUST-IN-TIME: K ct / Q (ct, qc) chains are injected between attention
     kt-slots of the preceding block, so the ACT engine starts exp-ing
     ~35us in and never drains.
  3. attention (qc, pt) blocks, head PAIRS: the two heads of a partition
     tile run their K=64 scoresT matmuls on PE row groups (0,0)/(64,0) via
     tile_position; one ACT instr does exp(s/8 + maskbias) for both heads;
     PV in transposed form outT[65, q] += V_h(+ones).T @ expT accumulates
     values + softmax denominators (ones column in an M=128 stationary
     window).  Per q-chunk: reciprocal on DVE (reciprocal_approx_fast),
     DRAM-bounce partition-broadcast, in-place normalize, then y = outT.T
     @ wo in 4 pieces -- all injected into later blocks' kt-slot gaps.
"""

import os
import sys

import ml_dtypes
import numpy as np

_TRN_REPO = "/opt/trn_rl_repo"
if _TRN_REPO not in sys.path:
    sys.path.insert(0, _TRN_REPO)

from contextlib import ExitStack

import concourse.bass as bass
import concourse.mybir as mybir
import concourse.tile as tile
from concourse import library_config
from concourse.bass_utils import run_bass_kernel_spmd

# If BASS_TRACE is set in the environment, run_bass_kernel_spmd imports
# antenv.axon_hooks, which this container image lacks -- pre-install a stub
# so kernel() degrades to an untraced run instead of crashing.  test.py
# overwrites the stub with a real ctypes-backed hook for profiling.
if "antenv.axon_hooks" not in sys.modules:
    try:
        import antenv.axon_hooks  # noqa: F401
    except Exception:
        import types as _types

        _hookmod = _types.ModuleType("antenv.axon_hooks")
        _hookstore = {}
        _hookmod.set_axon_ntff_profile_hook = lambda h: _hookstore.__setitem__(
            "h", h
        )
        _hookmod.get_axon_ntff_profile_hook = lambda: _hookstore.get("h")
        sys.modules["antenv.axon_hooks"] = _hookmod
        try:
            import antenv

            antenv.axon_hooks = _hookmod
        except Exception:
            pass

S, D, H, DK = 2048, 1024, 16, 64
NCORES = 8
HG = 2                # head-parallel groups
B = 4                 # batches
H8 = H // HG          # heads per core
C = H8 * DK           # 512: per-core projection width
P = 128
KT = D // P           # 8  k-tiles over D
ST = S // P           # 16 tiles over S
CT = C // P           # 4  tiles over C
VW = DK + 1           # 65: v columns + ones column
QC = 512              # q-chunk in attention phase (head-pair scheme)
NQC = S // QC

f32 = mybir.dt.float32
f32r = mybir.dt.float32r
bf16 = mybir.dt.bfloat16
i32 = mybir.dt.int32
FT = mybir.ActivationFunctionType
ALU = mybir.AluOpType


def build_nc(split_waits=True):
    nc = bass.Bass()
    xt_d = nc.declare_dram_parameter("xt", [D, S], bf16, isOutput=False)
    wq_d = nc.declare_dram_parameter("wq", [D, C], bf16, isOutput=False)
    wk_d = nc.declare_dram_parameter("wk", [D, C], bf16, isOutput=False)
    wv_d = nc.declare_dram_parameter("wv", [D, C], bf16, isOutput=False)
    wo_d = nc.declare_dram_parameter("wo", [C, D], bf16, isOutput=False)
    mask_d = nc.declare_dram_parameter("maskt", [P, ST], i32, isOutput=False)
    y_d = nc.declare_dram_parameter("y", [S, D], f32, isOutput=True)

    with tile.TileContext(nc) as tc, ExitStack() as ctx:
        perm = ctx.enter_context(tc.tile_pool(name="perm", bufs=1))

        # mask bias: (m - 1) * 1e9 per key, keys on partitions, one col per k-tile
        mask_i = perm.tile([P, ST], i32)
        nc.sync.dma_start(mask_i, mask_d[:, :])
        mask_b = perm.tile([P, ST], f32)
        nc.vector.tensor_copy(mask_b, mask_i)
        nc.vector.tensor_scalar(mask_b, mask_b, -1.0, 1.0e9, ALU.add, ALU.mult)

        # xT arrives in 8 S-chunks so the V projection can start after ~1/8
        # of the transfer (chunks land on different DMA queues).
        xT = perm.tile([P, KT, S], bf16)
        xt_r = xt_d.rearrange("(kt p) s -> p kt s", p=P)
        # wk first (the critical lead-in path), then the first xT s-range,
        # then the rest; everything in small chunks so they spread across
        # the 16 DMA queues (~11 GB/s each).
        wv_sb = perm.tile([P, KT, C], bf16)
        wq_sb = perm.tile([P, KT, C], bf16)
        wk_sb = perm.tile([P, KT, C], bf16)
        # critical prefix on disjoint queues: wk (8 chunks) || xT s0:512
        # (8 chunks) feed the first K/Q chains; then wq, wv, the rest of xT.
        wk_r = wk_d.rearrange("(kt p) c -> p kt c", p=P)
        for kt in range(KT):
            nc.sync.dma_start(
                wk_sb[:, kt : kt + 1, :], wk_r[:, kt : kt + 1, :]
            )
        for i in range(4):
            for j in range(2):
                nc.sync.dma_start(
                    xT[:, 2 * i : 2 * (i + 1), 256 * j : 256 * (j + 1)],
                    xt_r[:, 2 * i : 2 * (i + 1), 256 * j : 256 * (j + 1)],
                )
        wq_r = wq_d.rearrange("(kt p) c -> p kt c", p=P)
        for kt in range(KT):
            nc.sync.dma_start(
                wq_sb[:, kt : kt + 1, :], wq_r[:, kt : kt + 1, :]
            )
        wv_r = wv_d.rearrange("(kt p) c -> p kt c", p=P)
        for kt in range(KT):
            nc.sync.dma_start(
                wv_sb[:, kt : kt + 1, :], wv_r[:, kt : kt + 1, :]
            )
        for i in range(2):
            for j in range(2, 8):
                nc.sync.dma_start(
                    xT[:, 4 * i : 4 * (i + 1), 256 * j : 256 * (j + 1)],
                    xt_r[:, 4 * i : 4 * (i + 1), 256 * j : 256 * (j + 1)],
                )
        wo_sb = perm.tile([P, CT, D], bf16)
        wo_r = wo_d.rearrange("(pt p) e -> p pt e", p=P)
        for pt in range(CT):
            nc.sync.dma_start(wo_sb[:, pt : pt + 1, :], wo_r[:, pt : pt + 1, :])

        QT = perm.tile([P, CT, S], bf16)
        KTl = perm.tile([P, CT, S], bf16)
        # per-head PV stationary [V_h | ones]: M=65 keeps LDWEIGHTS short
        # (cost scales with stationary columns)
        V = perm.tile([P, ST, H8 * VW], bf16)
        V4 = V.rearrange("p st (h w) -> p st h w", w=VW)
        ones_sc = perm.tile([P, 1], f32)
        nc.vector.memset(ones_sc[:, :], 1.0)
        for st in range(ST):
            nc.vector.tensor_copy(
                V4[:, st, :, DK : DK + 1],
                ones_sc[:, :, None].to_broadcast((P, H8, 1)),
            )
        outT = perm.tile([P, CT, S], bf16)
        # 32 (head, q-chunk) row-sum vectors packed at start partitions
        # {0,32,64,96} x 8 column blocks (engine SBUF APs must start at k*32)
        rowsums = perm.tile([P, H8 * NQC // 4, QC], f32)
        nc.vector.memset(rowsums[:, :, :], 1.0)
        # bf16 copies of 1/rowsums (stationary/moving for the PE-matmul
        # partition-broadcast) + an all-ones bf16 column
        rowsums_bf = perm.tile([P, H8 * NQC // 4, QC], bf16)
        ones_bf = perm.tile([P, DK], bf16)
        nc.vector.tensor_copy(ones_bf, ones_sc.to_broadcast((P, DK)))

        with (
            tc.tile_pool(name="scps", bufs=2, space="PSUM") as scp,
            tc.tile_pool(name="otps", bufs=2, space="PSUM") as otp,
            tc.tile_pool(name="auxps", bufs=2, space="PSUM") as aux,
            tc.tile_pool(name="expool", bufs=5) as exp_pool,
            tc.tile_pool(name="ypool", bufs=3) as ypl,
        ):
            # warm the PE (HAM un-throttles after ~3.4us of activity) during
            # the input-DMA wait with throwaway matmuls on a zeroed tile
            warm = exp_pool.tile([P, 512], bf16, tag="warm")
            nc.gpsimd.memset(warm[:, :], 0.0)
            warm_ps = aux.tile([P, 512], f32, tag="mm", name="warmps")
            for _ in range(16):
                nc.tensor.matmul(
                    warm_ps, warm[:, 0:P], warm, start=True, stop=True
                )

            def v_block(st):
                ps = aux.tile([P, C], f32, tag="mm", name="psv")
                for kt in range(KT):
                    nc.tensor.matmul(
                        ps,
                        xT[:, kt, st * P : (st + 1) * P],
                        wv_sb[:, kt, :],
                        start=(kt == 0),
                        stop=(kt == KT - 1),
                    )
                nc.vector.tensor_copy(
                    V4[:, st, :, 0:DK],
                    ps.rearrange("p (h w) -> p h w", w=DK),
                )

            def proj_chain(w_sb, dst, ct, sch):
                ps = aux.tile([P, C], f32, tag="mm", name="psp")
                for kt in range(KT):
                    nc.tensor.matmul(
                        ps,
                        w_sb[:, kt, ct * P : (ct + 1) * P],
                        xT[:, kt, sch * 512 : (sch + 1) * 512],
                        start=(kt == 0),
                        stop=(kt == KT - 1),
                    )
                nc.vector.tensor_copy(
                    dst[:, ct, sch * 512 : (sch + 1) * 512], ps
                )

            def norm_half(qc, half):
                # 1/rowsums via exp(-ln(x)) on ACT (unused lanes hold 1.0),
                # partition-broadcast via K=1 PE matmuls (ones column x recip
                # row -> PSUM), then normalize 2 of the 4 partition tiles of
                # q-chunk qc of outT in place (half 0: heads 0-3, half 1:
                # heads 4-7 -- lets the last chunk's first half run early).
                qs = slice(qc * QC, (qc + 1) * QC)
                blk = 2 * qc + half
                rsp = rowsums[:, blk : blk + 1, :]
                nc.scalar.activation(rsp, rsp, FT.Ln)
                nc.scalar.activation(rsp, rsp, FT.Exp, scale=-1.0)
                nc.vector.tensor_copy(rowsums_bf[:, blk : blk + 1, :], rsp)
                for pt in (2 * half, 2 * half + 1):
                    bc = aux.tile([P, QC], f32, tag="mm", name="bc")
                    for hh in range(2):
                        h = 2 * pt + hh
                        r0 = (h % 4) * 32
                        nc.tensor.matmul(
                            bc[hh * DK : (hh + 1) * DK, :],
                            ones_bf[r0 : r0 + 1, :],
                            rowsums_bf[r0 : r0 + 1, blk, :],
                            start=True,
                            stop=True,
                            tile_position=(r0, hh * DK),
                        )
                    nc.vector.tensor_mul(outT[:, pt, qs], outT[:, pt, qs], bc)

            def y_piece(qc, sti):
                # one S-tile of y = outT.T @ wo; pt-outer so each outT
                # stationary serves both 512-wide output halves.
                st = qc * (QC // P) + sti
                y_sb = ypl.tile([P, D], f32, tag="y")
                pss = [
                    aux.tile([P, QC], f32, tag="mm", name=f"rsy{e}")
                    for e in range(2)
                ]
                for pt in range(CT):
                    for ec in range(2):
                        nc.tensor.matmul(
                            pss[ec],
                            outT[:, pt, st * P : (st + 1) * P],
                            wo_sb[:, pt, ec * 512 : (ec + 1) * 512],
                            start=(pt == 0),
                            stop=(pt == CT - 1),
                        )
                for ec in range(2):
                    nc.vector.tensor_copy(
                        y_sb[:, ec * 512 : (ec + 1) * 512], pss[ec]
                    )
                    nc.sync.dma_start(
                        y_d[st * P : (st + 1) * P, ec * 512 : (ec + 1) * 512],
                        y_sb[:, ec * 512 : (ec + 1) * 512],
                    )

            # cross-block software pipeline: each slot's PV pair is deferred
            # until after the NEXT slot's scores+exp, so at block boundaries
            # the next block's first exp is not stuck behind PVs that wait
            # on this block's last exp.
            pending_pv = [None]

            def flush_pv():
                if pending_pv[0] is not None:
                    fn, pending_pv[0] = pending_pv[0], None
                    fn()

            def att_block(qc, pt, work):
                # 16 kt-slots of scores-pair -> exp -> PV(prev slot); `work`
                # maps kt-slot -> closures (projection chains / V blocks /
                # norm / y pieces for neighboring blocks) injected so the PE
                # executes them inside the ACT-bound exp stream.
                qs = slice(qc * QC, (qc + 1) * QC)
                h0, h1 = 2 * pt, 2 * pt + 1
                ot0 = otp.tile([VW, QC], f32, tag="ot", name="ot0")
                ot1 = otp.tile([VW, QC], f32, tag="ot", name="ot1")

                def drain():
                    # rowsum vector (h, qc) at row (h%4)*32, block qc*2+h//4
                    for half, ot in ((0, ot0), (1, ot1)):
                        h = 2 * pt + half
                        nc.vector.tensor_copy(
                            rowsums[
                                (h % 4) * 32 : (h % 4) * 32 + 1,
                                2 * qc + h // 4,
                                :,
                            ],
                            ot[DK : DK + 1, :],
                        )
                        nc.vector.tensor_copy(
                            outT[half * DK : (half + 1) * DK, pt, qs],
                            ot[0:DK, :],
                        )

                for kt in range(ST):
                    for fn in work.get(kt, ()):
                        fn()
                    sc_ps = scp.tile([P, 2, QC], f32, tag="sc")
                    nc.tensor.matmul(
                        sc_ps[:, 0, :],
                        KTl[0:DK, pt, kt * P : (kt + 1) * P],
                        QT[0:DK, pt, qs],
                        start=True,
                        stop=True,
                        tile_position=(0, 0),
                    )
                    nc.tensor.matmul(
                        sc_ps[:, 1, :],
                        KTl[DK:P, pt, kt * P : (kt + 1) * P],
                        QT[DK:P, pt, qs],
                        start=True,
                        stop=True,
                        tile_position=(64, 0),
                    )
                    ex = exp_pool.tile([P, 2, QC], bf16, tag="ex")
                    nc.scalar.activation(
                        ex.rearrange("p a b -> p (a b)"),
                        sc_ps.rearrange("p a b -> p (a b)"),
                        FT.Exp,
                        bias=mask_b[:, kt : kt + 1],
                    )
                    flush_pv()

                    def pv(kt=kt, ex=ex):
                        nc.tensor.matmul(
                            ot0,
                            V4[:, kt, h0, :],
                            ex[:, 0, :],
                            start=(kt == 0),
                            stop=(kt == ST - 1),
                        )
                        nc.tensor.matmul(
                            ot1,
                            V4[:, kt, h1, :],
                            ex[:, 1, :],
                            start=(kt == 0),
                            stop=(kt == ST - 1),
                        )
                        if kt == ST - 1:
                            drain()

                    pending_pv[0] = pv

            # ---- attention with just-in-time V/K/Q projections + y.
            # Lead-in is only K(ct0, sch0) + Q(0, 0): the first attention
            # block's early kt-slots run on those while everything else
            # (V blocks, remaining K/Q chains) is injected into slot gaps.
            def K_chain(ct, sch):
                return lambda: proj_chain(wk_sb, KTl, ct, sch)

            def Q_chain(ct, qc):
                return lambda: proj_chain(wq_sb, QT, ct, qc)

            def V_blk(st):
                return lambda: v_block(st)

            proj_chain(wk_sb, KTl, 0, 0)
            proj_chain(wq_sb, QT, 0, 0)

            for qc in range(NQC):
                for pt in range(CT):
                    work = {}

                    def add(kt, fn):
                        work.setdefault(kt, []).append(fn)

                    if qc == 0:
                        if pt == 0:
                            # V tile st must land before its PV, which is
                            # deferred past exp(st+1): inject one slot later
                            # so each slot's scores lead the PE queue
                            for st in range(ST):
                                add(min(st + 1, ST - 1), V_blk(st))
                            add(1, K_chain(0, 1))
                            add(5, K_chain(0, 2))
                            add(9, K_chain(0, 3))
                            add(12, K_chain(1, 0))
                            add(14, Q_chain(1, 0))
                        elif pt < 3:
                            add(1, K_chain(pt, 1))
                            add(4, K_chain(pt, 2))
                            add(7, K_chain(pt, 3))
                            add(10, K_chain(pt + 1, 0))
                            add(13, Q_chain(pt + 1, 0))
                        else:
                            add(1, K_chain(3, 1))
                            add(4, K_chain(3, 2))
                            add(7, K_chain(3, 3))
                            add(11, Q_chain(0, 1))
                    else:
                        if pt == 0:
                            add(1, lambda q=qc - 1: norm_half(q, 0))
                            add(3, lambda q=qc - 1: norm_half(q, 1))
                            add(8, lambda q=qc - 1: y_piece(q, 0))
                            add(12, Q_chain(1, qc))
                        elif pt == 1:
                            add(2, lambda q=qc - 1: y_piece(q, 1))
                            add(8, lambda q=qc - 1: y_piece(q, 2))
                            add(12, Q_chain(2, qc))
                        elif pt == 2:
                            add(2, lambda q=qc - 1: y_piece(q, 3))
                            if qc == NQC - 1:
                                # last chunk's heads 0-3 normalize early
                                # (their rowsums completed with block pt1)
                                add(6, lambda: norm_half(NQC - 1, 0))
                            add(12, Q_chain(3, qc))
                        elif qc < NQC - 1:
                            add(8, Q_chain(0, qc + 1))
                    att_block(qc, pt, work)

            flush_pv()
            norm_half(NQC - 1, 1)
            for sti in range(4):
                y_piece(NQC - 1, sti)

    if split_waits:
        _split_matmul_waits(nc)
    return nc


def _split_matmul_waits(nc):
    """fp32/f32r matmuls (and DMA descriptors) lower to structs that hold
    only ONE sync wait; move extra waits onto a nop on the same engine."""
    import bass_rust

    n = 0
    for f in nc.m.functions:
        for blk in f.blocks:
            out = []
            for inst in blk.instructions:
                si = getattr(inst, "sync_info", None)
                if si is not None and len(si.on_wait) > 1:
                    waits = list(si.on_wait)
                    for w in waits[:-1]:
                        nop = bass_rust.InstNoOp(
                            name=f"I-mmw{n}", ins=[], outs=[], engine=inst.engine
                        )
                        n += 1
                        nop.sync_info = bass_rust.SyncInfo(
                            on_wait=[w], on_update=[]
                        )
                        out.append(nop)
                    inst.sync_info = bass_rust.SyncInfo(
                        on_wait=waits[-1:], on_update=list(si.on_update)
                    )
                out.append(inst)
            blk.instructions = out
    return nc


_NC_CACHE = None


def get_nc():
    global _NC_CACHE
    if _NC_CACHE is None:
        _NC_CACHE = build_nc()
    return _NC_CACHE


def make_in_maps(inputs):
    inp = np.asarray(inputs["inputs"], dtype=np.float32)
    mask = np.asarray(inputs["mask"], dtype=np.int32)
    Wq = np.asarray(inputs["Wq"], dtype=np.float32)
    Wk = np.asarray(inputs["Wk"], dtype=np.float32)
    Wv = np.asarray(inputs["Wv"], dtype=np.float32)
    Wo = np.asarray(inputs["Wo"], dtype=np.float32)
    b16 = ml_dtypes.bfloat16

    in_maps = []
    for c in range(NCORES):
        b, g = c // HG, c % HG
        cs = slice(g * C, (g + 1) * C)
        in_maps.append(
            {
                "xt": np.ascontiguousarray(inp[b].T.astype(b16)),
                "wq": np.ascontiguousarray((Wq[:, cs] * 0.125).astype(b16)),
                "wk": np.ascontiguousarray(Wk[:, cs].astype(b16)),
                "wv": np.ascontiguousarray(Wv[:, cs].astype(b16)),
                "wo": np.ascontiguousarray(Wo[cs, :].astype(b16)),
                "maskt": np.ascontiguousarray(mask[b].reshape(ST, P).T),
            }
        )
    return in_maps


def gather(results):
    out = np.empty((B, S, D), np.float32)
    for b in range(B):
        out[b] = results[HG * b]["y"] + results[HG * b + 1]["y"]
    return out


def run(inputs, **kwargs):
    """Run on hardware; returns (output, BassKernelResults)."""
    res = run_bass_kernel_spmd(
        get_nc(), make_in_maps(inputs), list(range(NCORES)), **kwargs
    )
    return gather(res.results), res


def kernel(**inputs) -> np.ndarray:
    out, _ = run(inputs)
    return out
